# revision 1
# baseline (speedup 1.0000x reference)
"""GatedAttention Trainium2 kernel, 8-way tensor-parallel over heads.

Reference computation (B=1, S=2048, D=2048, H=16 heads, Hd=128):
  q,k,v = x @ {q,k,v}_w.T  (per-head split)
  scores = (q @ k.T) / sqrt(Hd), causal mask, softmax
  av = attn @ v
  gate = sigmoid(q @ gate_w.T + gate_b)       (per-head)
  y = concat_heads(av * gate) @ o_w.T

Sharding: 2 heads per core (column-parallel QKV/gate). The gated per-head
outputs are AllGathered in bf16 [feature, seq] layout — one AllGather per
local head so the first overlaps the second head's attention and the second
overlaps the first half of o_proj. o_proj is column-parallel; the host
concatenates the 8 output column slices.

All matmuls run on the PE in bf16 with fp32 PSUM accumulation. Softmax runs
without max-subtraction (scores are small by construction); exp row-sums
ride on the PE as M=1 ones-matmuls in the same transposed [j, q] layout, so
no on-chip transposes are needed anywhere. Gate sigmoids are all computed
before attention so the ACT engine loads each activation table once.
"""

import numpy as np
import ml_dtypes

import concourse.bass as bass
import concourse.mybir as mybir
import concourse.tile as tile
from concourse import bacc
from concourse.bass_utils import run_bass_kernel_spmd

BF16 = ml_dtypes.bfloat16
F32 = mybir.dt.float32
BF = mybir.dt.bfloat16
AF = mybir.ActivationFunctionType

N_CORES = 8
S = 2048          # sequence length
D = 2048          # model dim
H = 16            # total heads
HD = 128          # head dim
HPC = H // N_CORES                   # heads per core: 2
E = HPC * HD                         # 256 output dims per core
DC = D // 128                        # 16 contraction chunks
QCW = 512                            # q-chunk width
NQC = S // QCW                       # 4 q-chunks
SCALE = 1.0 / float(np.sqrt(HD))

_CACHED = {}


def _build(collective=True):
    nc = bacc.Bacc("TRN2", target_bir_lowering=False, debug=False,
                   num_devices=N_CORES if collective else 1,
                   enable_asserts=False)

    xt = nc.dram_tensor("xt", [D, S], BF, kind="ExternalInput")        # x^T
    wqt = nc.dram_tensor("wqt", [D, E], BF, kind="ExternalInput")      # q_w shard^T
    wkt = nc.dram_tensor("wkt", [D, E], BF, kind="ExternalInput")
    wvt = nc.dram_tensor("wvt", [D, E], BF, kind="ExternalInput")
    owt = nc.dram_tensor("owt", [D, E], BF, kind="ExternalInput")      # o_w shard^T
    gwt = nc.dram_tensor("gwt", [HD, HD], BF, kind="ExternalInput")    # gate_w^T
    gb = nc.dram_tensor("gb", [HD, 1], F32, kind="ExternalInput")      # gate bias
    trim = nc.dram_tensor("trim", [128, 128], BF, kind="ExternalInput")
    yt = nc.dram_tensor("yt", [E, S], F32, kind="ExternalOutput")      # y^T slice

    shared = "Shared" if collective else "Local"

    with tile.TileContext(nc) as tc:
        with tc.tile_pool(name="const", bufs=1) as const, \
             tc.tile_pool(name="work", bufs=2) as work, \
             tc.tile_pool(name="psum", bufs=1, space="PSUM") as psum, \
             tc.tile_pool(name="dram", bufs=1, space="DRAM") as dram:

            def pp(name):
                return psum.tile([128, QCW], F32, tag="pp", bufs=8, name=name)

            # ---- input loads (few big DMAs; xts chunked to feed the
            #      dc-synchronized projection loop) ----
            wqts = const.tile([128, DC, E], BF, tag="wqts", name="wqts")
            wkts = const.tile([128, DC, E], BF, tag="wkts", name="wkts")
            xts = const.tile([128, DC, S], BF, tag="big", name="xts")

            def _ldw(dst, src, half):
                sl = slice(half * 8, (half + 1) * 8)
                nc.sync.dma_start(
                    dst[:, sl, :],
                    src.ap()[half * 1024:(half + 1) * 1024, :]
                       .rearrange("(c p) e -> p c e", p=128))

            # interleave weight halves with the x chunks so the transfer
            # stream stays just ahead of group A's dc-ordered consumption
            def _ldx(d0, d1):
                nc.sync.dma_start(
                    xts[:, d0:d1, :],
                    xt.ap()[d0 * 128:d1 * 128, :]
                      .rearrange("(c p) s -> p c s", p=128))

            _ldw(wqts, wqt, 0)
            _ldx(0, 1)
            _ldw(wkts, wkt, 0)
            for d in range(1, 8):
                _ldx(d, d + 1)
            _ldw(wqts, wqt, 1)
            _ldx(8, 9)
            _ldw(wkts, wkt, 1)
            _ldx(9, 10)
            for k in range(5, 8):
                _ldx(2 * k, 2 * k + 2)

            gwts = const.tile([HD, HD], BF, tag="gwts", name="gwts")
            gbs = const.tile([HD, 1], F32, tag="gbs", name="gbs")
            tris = const.tile([128, 128], BF, tag="tris", name="tris")
            ones128 = const.tile([128, 1], BF, tag="ones128", name="ones128")
            one1 = const.tile([1, 128], F32, tag="one1", name="one1")
            nc.sync.dma_start(gwts[:], gwt.ap())
            nc.sync.dma_start(gbs[:], gb.ap())
            nc.sync.dma_start(tris[:], trim.ap())
            nc.vector.memset(ones128[:], 1.0)
            nc.vector.memset(one1[:], 1.0)

            wvts = const.tile([128, DC, E], BF, tag="wvts", name="wvts")
            nc.sync.dma_start(wvts[:], wvt.ap().rearrange("(c p) e -> p c e", p=128))

            # ---- projections ----
            # Q^T, K^T: [e(2x128), s].  Groups of 8 PSUM banks, dc-inner so
            # PE work tracks the streaming xts chunks.
            qts = const.tile([128, HPC, S], BF, tag="qts", name="qts")
            kts = const.tile([128, HPC, S], BF, tag="kts", name="kts")

            # ec=0: dc-inner across 8 psums so PE work tracks streaming xts
            # chunks.  ec=1: slot-major (xts resident), each chain overlaps
            # the previous psum's copy.
            qps = [pp("qp") for _ in range(NQC)]
            kps = [pp("kp") for _ in range(NQC)]
            for dc in range(DC):
                st = (dc == 0)
                sp = (dc == DC - 1)
                for sc in range(NQC):
                    nc.tensor.matmul(
                        qps[sc][:], wqts[:, dc, 0:128],
                        xts[:, dc, sc * QCW:(sc + 1) * QCW], start=st, stop=sp)
                for sc in range(NQC):
                    nc.tensor.matmul(
                        kps[sc][:], wkts[:, dc, 0:128],
                        xts[:, dc, sc * QCW:(sc + 1) * QCW], start=st, stop=sp)
            for sc in range(NQC):
                nc.vector.tensor_copy(
                    out=qts[:, 0, sc * QCW:(sc + 1) * QCW], in_=qps[sc][:])
                nc.vector.tensor_copy(
                    out=kts[:, 0, sc * QCW:(sc + 1) * QCW], in_=kps[sc][:])
            for wts, outts in ((wqts, qts), (wkts, kts)):
                for sc in range(NQC):
                    ppt = pp("qp")
                    for dc in range(DC):
                        nc.tensor.matmul(
                            ppt[:], wts[:, dc, 128:256],
                            xts[:, dc, sc * QCW:(sc + 1) * QCW],
                            start=(dc == 0), stop=(dc == DC - 1))
                    nc.vector.tensor_copy(
                        out=outts[:, 1, sc * QCW:(sc + 1) * QCW], in_=ppt[:])

            # o_proj weights: reuse the wqts slot (dead after the loop above)
            owts = const.tile([128, DC, E], BF, tag="wqts", name="owts")
            nc.sync.dma_start(owts[:], owt.ap().rearrange("(c p) e -> p c e", p=128))

            # gates for both heads, before the V projection so the sigmoid
            # table load and ACT latency hide behind V's matmuls
            gts = const.tile([128, HPC, S], BF, tag="gts", name="gts")
            for h in range(HPC):
                for qc in range(NQC):
                    gp = pp("gp")
                    nc.tensor.matmul(gp[:], gwts[:],
                                     qts[:, h, qc * QCW:(qc + 1) * QCW],
                                     start=True, stop=True)
                    nc.scalar.activation(gts[:, h, qc * QCW:(qc + 1) * QCW],
                                         gp[:], AF.Sigmoid, bias=gbs[:, 0:1])

            # V: [s(16x128), e] natural layout.  Slot-major (xts is fully
            # resident by now): each psum's 16-matmul chain runs while the
            # previous psum's copy drains, so group boundaries don't stall.
            vts = const.tile([128, DC, E], BF, tag="vts", name="vts")
            for sc16 in range(DC):
                vp = pp("vp")
                for dc in range(DC):
                    nc.tensor.matmul(
                        vp[:, :E],
                        xts[:, dc, sc16 * 128:(sc16 + 1) * 128],
                        wvts[:, dc, :], start=(dc == 0), stop=(dc == DC - 1))
                nc.vector.tensor_copy(out=vts[:, sc16, :], in_=vp[:, :E])

            # ---- attention (transposed layout), AllGather per head ----
            attds = [dram.tile([HD, S], BF, tag=f"attd{h}", name=f"attd{h}")
                     for h in range(HPC)]
            outds = [dram.tile([N_CORES * HD, S], BF, tag=f"outd{h}",
                               addr_space=shared, name=f"outd{h}")
                     for h in range(HPC)]

            # Software-pipelined across (h, qc) blocks: each block's last
            # AV/sums matmuls and its epilogue are emitted after the NEXT
            # block's first scores/exp, so the PE never idles waiting for
            # the tail exp on ACT.
            def emit_ag(h):
                if collective:
                    nc.gpsimd.collective_compute(
                        "AllGather", mybir.AluOpType.bypass,
                        replica_groups=[list(range(N_CORES))],
                        ins=[attds[h][:].opt()], outs=[outds[h][:].opt()])
                else:
                    nc.sync.dma_start(outds[h][0:HD, :], attds[h][:])
                if h == 0:
                    # prefetch gathered even-f-chunk features during the
                    # second head's attention (reuses the dead xts slot);
                    # chunked so o_proj can start after the first slice
                    out0 = const.tile([128, N_CORES, S], BF, tag="big",
                                      name="out0")
                    for sc in range(NQC):
                        nc.sync.dma_start(
                            out0[:, :, sc * QCW:(sc + 1) * QCW],
                            outds[0][:, sc * QCW:(sc + 1) * QCW]
                                .rearrange("(g p) s -> p g s", p=128))
                    return out0
                return None

            out0 = None
            pend = None   # deferred tail of the previous block

            def emit_tail_av(t, k):
                # deferred AV/sums for jj_l-1 (k=0) or jj_l (k=1, stop)
                (h, q0, avp, sump, exts_l, s0s, jj_l) = t
                jj = jj_l - 1 + k
                s0 = s0s[k]
                nc.tensor.matmul(
                    avp[:, s0:], vts[:, jj, h * 128:(h + 1) * 128],
                    exts_l[jj % 3][:, s0:], start=False, stop=(k == 1))
                nc.tensor.matmul(
                    sump[:, s0:], ones128[:], exts_l[jj % 3][:, s0:],
                    start=False, stop=(k == 1))

            def emit_tail(t):
                nonlocal out0
                (h, q0, avp, sump, exts_l, s0s, jj_l) = t
                rs = work.tile([1, QCW], F32, tag="rs", bufs=2, name="rs")
                nc.vector.reciprocal(out=rs[:], in_=sump[:])
                # broadcast 1/sum across partitions on the (idle) Pool engine
                # so the epilogue never blocks the PE
                bcb = work.tile([128, QCW], F32, tag="bcb", bufs=2, name="bcb")
                nc.gpsimd.partition_broadcast(bcb[:], rs[:])
                gn = work.tile([128, QCW], BF, tag="gn", bufs=2, name="gn")
                nc.vector.tensor_mul(gn[:], gts[:, h, q0:q0 + QCW], bcb[:])
                att = work.tile([128, QCW], BF, tag="att", bufs=2, name="att")
                nc.vector.tensor_mul(att[:], avp[:], gn[:])
                nc.sync.dma_start(attds[h][:, q0:q0 + QCW], att[:])
                if q0 == (NQC - 1) * QCW:
                    o = emit_ag(h)
                    if o is not None:
                        out0 = o

            for h in range(HPC):
                for qc in range(NQC):
                    q0 = qc * QCW
                    scps = [pp("scp") for _ in range(3)]
                    avp = pp("avp")
                    sump = psum.tile([1, QCW], F32, tag="pp", bufs=8, name="sump")
                    njj = 4 * qc + 4
                    exts = [work.tile([128, QCW], BF, tag="ext", bufs=6,
                                      name="ext") for _ in range(3)]
                    def s0_of(jj):
                        return max(0, (jj - 4 * qc) * 128)

                    def emit_av(jj):
                        s0 = s0_of(jj)
                        nc.tensor.matmul(
                            avp[:, s0:], vts[:, jj, h * 128:(h + 1) * 128],
                            exts[jj % 3][:, s0:],
                            start=(jj == 0), stop=False)
                        nc.tensor.matmul(
                            sump[:, s0:], ones128[:], exts[jj % 3][:, s0:],
                            start=(jj == 0), stop=False)

                    # scores run two jj ahead of AV/sums so the PE never
                    # waits on the exp->mask chain; the last block's two
                    # deferred AV/sums pairs land in this block's jj=0/1
                    for jj in range(njj):
                        off = jj - 4 * qc
                        s0 = s0_of(jj)
                        scp = scps[jj % 3]
                        ext = exts[jj % 3]
                        nc.tensor.matmul(
                            scp[:, s0:], kts[:, h, jj * 128:(jj + 1) * 128],
                            qts[:, h, q0 + s0:q0 + QCW], start=True, stop=True)
                        nc.scalar.activation(ext[:, s0:], scp[:, s0:],
                                             AF.Exp, scale=SCALE)
                        if off >= 0:
                            nc.vector.tensor_mul(ext[:, s0:s0 + 128],
                                                 ext[:, s0:s0 + 128], tris[:])
                        if pend is not None:
                            if jj == 0:
                                emit_tail_av(pend, 0)
                            elif jj == 1:
                                emit_tail_av(pend, 1)
                                emit_tail(pend)
                                pend = None
                        if jj >= 2:
                            emit_av(jj - 2)
                    pend = (h, q0, avp, sump, exts,
                            (s0_of(njj - 2), s0_of(njj - 1)), njj - 1)
                # flush at the head boundary so the head's last store — and
                # with it the AllGather — issues as early as possible
                emit_tail_av(pend, 0)
                emit_tail_av(pend, 1)
                emit_tail(pend)
                pend = None

            # ---- o_proj: y^T[e', s] = sum_f o_w[cs+e', f] out^T[f, s] ----
            # out0 covers even global f-chunks (2g), out1 odd (2g+1).
            # yp pairs are allocated per-sc so the first A matmuls only wait
            # on two PSUM slots, not on the whole attention drain.
            yps = []
            for sc in range(NQC):
                yps.append([pp("yp") for _ in range(HPC)])
                for ec in range(HPC):
                    for g in range(N_CORES):
                        nc.tensor.matmul(
                            yps[sc][ec][:],
                            owts[:, 2 * g, ec * 128:(ec + 1) * 128],
                            out0[:, g, sc * QCW:(sc + 1) * QCW],
                            start=(g == 0), stop=False)

            out1 = const.tile([128, N_CORES, S], BF, tag="out1", name="out1")
            for sc in range(NQC):
                nc.sync.dma_start(
                    out1[:, :, sc * QCW:(sc + 1) * QCW],
                    outds[1][:, sc * QCW:(sc + 1) * QCW]
                        .rearrange("(g p) s -> p g s", p=128))
            for sc in range(NQC):
                for ec in range(HPC):
                    for g in range(N_CORES):
                        nc.tensor.matmul(
                            yps[sc][ec][:],
                            owts[:, 2 * g + 1, ec * 128:(ec + 1) * 128],
                            out1[:, g, sc * QCW:(sc + 1) * QCW],
                            start=False, stop=(g == N_CORES - 1))
                for ec in range(HPC):
                    ys = work.tile([128, QCW], F32, tag="ys", bufs=4, name="ys")
                    nc.vector.tensor_copy(out=ys[:], in_=yps[sc][ec][:])
                    nc.sync.dma_start(
                        yt.ap()[ec * 128:(ec + 1) * 128, sc * QCW:(sc + 1) * QCW],
                        ys[:])

    nc.compile()
    return nc


def _prep_inputs(x, q_w, k_w, v_w, o_w, gate_w, gate_b):
    x = np.asarray(x, dtype=np.float32)
    xt = np.ascontiguousarray(x.reshape(S, D).T).astype(BF16)
    gwt = np.ascontiguousarray(np.asarray(gate_w, np.float32).T).astype(BF16)
    gb = np.asarray(gate_b, np.float32).reshape(HD, 1).copy()
    trim = np.triu(np.ones((128, 128), np.float32)).astype(BF16)
    in_maps = []
    for c in range(N_CORES):
        sl = slice(c * E, (c + 1) * E)
        in_maps.append({
            "xt": xt,
            "wqt": np.ascontiguousarray(np.asarray(q_w, np.float32)[sl, :].T).astype(BF16),
            "wkt": np.ascontiguousarray(np.asarray(k_w, np.float32)[sl, :].T).astype(BF16),
            "wvt": np.ascontiguousarray(np.asarray(v_w, np.float32)[sl, :].T).astype(BF16),
            "owt": np.ascontiguousarray(np.asarray(o_w, np.float32)[sl, :].T).astype(BF16),
            "gwt": gwt,
            "gb": gb,
            "trim": trim,
        })
    return in_maps


def _run(in_maps, **kwargs):
    if "nc" not in _CACHED:
        _CACHED["nc"] = _build()
    return run_bass_kernel_spmd(_CACHED["nc"], in_maps,
                                core_ids=list(range(N_CORES)), **kwargs)


def kernel(x, q_w, k_w, v_w, o_w, gate_w, gate_b):
    res = _run(_prep_inputs(x, q_w, k_w, v_w, o_w, gate_w, gate_b))
    yts = [res.results[c]["yt"] for c in range(N_CORES)]
    y_t = np.concatenate(yts, axis=0)          # [D(e), S]
    return np.ascontiguousarray(y_t.T, dtype=np.float32).reshape(1, S, D)



# revision 2
# speedup vs baseline: 1.0350x; 1.0350x over previous
"""GatedAttention Trainium2 kernel, 8-way tensor-parallel over heads.

Reference computation (B=1, S=2048, D=2048, H=16 heads, Hd=128):
  q,k,v = x @ {q,k,v}_w.T  (per-head split)
  scores = (q @ k.T) / sqrt(Hd), causal mask, softmax
  av = attn @ v
  gate = sigmoid(q @ gate_w.T + gate_b)       (per-head)
  y = concat_heads(av * gate) @ o_w.T

Sharding: 2 heads per core (column-parallel QKV/gate), o_proj ROW-parallel:
each core contracts its own 256 attention-output features against the
matching o_w columns and writes a full [S, D] fp32 partial; the host sums
the 8 partials. No cross-core collectives anywhere, so each core's NEFF
span contains only its own work — immune to launch skew and collective
stalls on the other cores.

All matmuls run on the PE in bf16 with fp32 PSUM accumulation. Softmax runs
without max-subtraction (scores are small by construction); exp row-sums
ride on the PE as M=1 ones-matmuls in the same transposed [j, q] layout, so
no on-chip transposes are needed anywhere. Gate sigmoids are all computed
before attention so the ACT engine loads each activation table once.
o_proj is emitted per q-chunk right after that chunk's attention epilogue,
spreading the 16MB output DMA across the attention stream.
"""

import numpy as np
import ml_dtypes

import concourse.bass as bass
import concourse.mybir as mybir
import concourse.tile as tile
from concourse import bacc
from concourse.bass_utils import run_bass_kernel_spmd

BF16 = ml_dtypes.bfloat16
F32 = mybir.dt.float32
BF = mybir.dt.bfloat16
AF = mybir.ActivationFunctionType

N_CORES = 8
S = 2048          # sequence length
D = 2048          # model dim
H = 16            # total heads
HD = 128          # head dim
HPC = H // N_CORES                   # heads per core: 2
E = HPC * HD                         # 256 local features per core
DC = D // 128                        # 16 contraction chunks
QCW = 512                            # q-chunk width
NQC = S // QCW                       # 4 q-chunks
NEC = D // QCW                       # 4 o_proj output column chunks
SCALE = 1.0 / float(np.sqrt(HD))

_CACHED = {}


def _build():
    nc = bacc.Bacc("TRN2", target_bir_lowering=False, debug=False,
                   num_devices=1, enable_asserts=False)

    xt = nc.dram_tensor("xt", [D, S], BF, kind="ExternalInput")        # x^T
    wqt = nc.dram_tensor("wqt", [D, E], BF, kind="ExternalInput")      # q_w shard^T
    wkt = nc.dram_tensor("wkt", [D, E], BF, kind="ExternalInput")
    wvt = nc.dram_tensor("wvt", [D, E], BF, kind="ExternalInput")
    owt2 = nc.dram_tensor("owt2", [E, D], BF, kind="ExternalInput")    # o_w cols^T
    gwt = nc.dram_tensor("gwt", [HD, HD], BF, kind="ExternalInput")    # gate_w^T
    gb = nc.dram_tensor("gb", [HD, 1], F32, kind="ExternalInput")      # gate bias
    trim = nc.dram_tensor("trim", [128, 128], BF, kind="ExternalInput")
    yt = nc.dram_tensor("yt", [S, D], F32, kind="ExternalOutput")      # partial y

    with tile.TileContext(nc) as tc:
        with tc.tile_pool(name="const", bufs=1) as const, \
             tc.tile_pool(name="work", bufs=2) as work, \
             tc.tile_pool(name="psum", bufs=1, space="PSUM") as psum:

            def pp(name):
                return psum.tile([128, QCW], F32, tag="pp", bufs=8, name=name)

            # ---- input loads (few big DMAs; xts chunked to feed the
            #      dc-synchronized projection loop) ----
            wqts = const.tile([128, DC, E], BF, tag="wqts", name="wqts")
            wkts = const.tile([128, DC, E], BF, tag="wkts", name="wkts")
            xts = const.tile([128, DC, S], BF, tag="big", name="xts")

            def _ldw(dst, src, half):
                sl = slice(half * 8, (half + 1) * 8)
                nc.sync.dma_start(
                    dst[:, sl, :],
                    src.ap()[half * 1024:(half + 1) * 1024, :]
                       .rearrange("(c p) e -> p c e", p=128))

            # interleave weight halves with the x chunks so the transfer
            # stream stays just ahead of group A's dc-ordered consumption
            def _ldx(d0, d1):
                nc.sync.dma_start(
                    xts[:, d0:d1, :],
                    xt.ap()[d0 * 128:d1 * 128, :]
                      .rearrange("(c p) s -> p c s", p=128))

            _ldw(wqts, wqt, 0)
            _ldx(0, 1)
            _ldw(wkts, wkt, 0)
            for d in range(1, 8):
                _ldx(d, d + 1)
            _ldw(wqts, wqt, 1)
            _ldx(8, 9)
            _ldw(wkts, wkt, 1)
            _ldx(9, 10)
            for k in range(5, 8):
                _ldx(2 * k, 2 * k + 2)

            gwts = const.tile([HD, HD], BF, tag="gwts", name="gwts")
            gbs = const.tile([HD, 1], F32, tag="gbs", name="gbs")
            tris = const.tile([128, 128], BF, tag="tris", name="tris")
            ones128 = const.tile([128, 1], BF, tag="ones128", name="ones128")
            nc.sync.dma_start(gwts[:], gwt.ap())
            nc.sync.dma_start(gbs[:], gb.ap())
            nc.sync.dma_start(tris[:], trim.ap())
            nc.vector.memset(ones128[:], 1.0)

            wvts = const.tile([128, DC, E], BF, tag="wvts", name="wvts")
            nc.sync.dma_start(wvts[:], wvt.ap().rearrange("(c p) e -> p c e", p=128))

            # ---- projections ----
            # Q^T, K^T: [e(2x128), s].  Groups of 8 PSUM banks, dc-inner so
            # PE work tracks the streaming xts chunks.
            qts = const.tile([128, HPC, S], BF, tag="qts", name="qts")
            kts = const.tile([128, HPC, S], BF, tag="kts", name="kts")

            # ec=0: dc-inner across 8 psums so PE work tracks streaming xts
            # chunks.  ec=1: slot-major (xts resident), each chain overlaps
            # the previous psum's copy.
            qps = [pp("qp") for _ in range(NQC)]
            kps = [pp("kp") for _ in range(NQC)]
            for dc in range(DC):
                st = (dc == 0)
                sp = (dc == DC - 1)
                for sc in range(NQC):
                    nc.tensor.matmul(
                        qps[sc][:], wqts[:, dc, 0:128],
                        xts[:, dc, sc * QCW:(sc + 1) * QCW], start=st, stop=sp)
                for sc in range(NQC):
                    nc.tensor.matmul(
                        kps[sc][:], wkts[:, dc, 0:128],
                        xts[:, dc, sc * QCW:(sc + 1) * QCW], start=st, stop=sp)
            for sc in range(NQC):
                nc.vector.tensor_copy(
                    out=qts[:, 0, sc * QCW:(sc + 1) * QCW], in_=qps[sc][:])
                nc.vector.tensor_copy(
                    out=kts[:, 0, sc * QCW:(sc + 1) * QCW], in_=kps[sc][:])
            for wts, outts in ((wqts, qts), (wkts, kts)):
                for sc in range(NQC):
                    ppt = pp("qp")
                    for dc in range(DC):
                        nc.tensor.matmul(
                            ppt[:], wts[:, dc, 128:256],
                            xts[:, dc, sc * QCW:(sc + 1) * QCW],
                            start=(dc == 0), stop=(dc == DC - 1))
                    nc.vector.tensor_copy(
                        out=outts[:, 1, sc * QCW:(sc + 1) * QCW], in_=ppt[:])

            # o_proj weights (row-parallel slice): [f(2x128), e(2048)]
            owts2 = const.tile([128, HPC, D], BF, tag="owts2", name="owts2")
            nc.sync.dma_start(owts2[:], owt2.ap().rearrange("(c p) e -> p c e", p=128))

            # gates for both heads, before the V projection so the sigmoid
            # table load and ACT latency hide behind V's matmuls
            gts = const.tile([128, HPC, S], BF, tag="gts", name="gts")
            for h in range(HPC):
                for qc in range(NQC):
                    gp = pp("gp")
                    nc.tensor.matmul(gp[:], gwts[:],
                                     qts[:, h, qc * QCW:(qc + 1) * QCW],
                                     start=True, stop=True)
                    nc.scalar.activation(gts[:, h, qc * QCW:(qc + 1) * QCW],
                                         gp[:], AF.Sigmoid, bias=gbs[:, 0:1])

            # V: [s(16x128), e] natural layout.  Slot-major (xts is fully
            # resident by now): each psum's 16-matmul chain runs while the
            # previous psum's copy drains, so group boundaries don't stall.
            vts = const.tile([128, DC, E], BF, tag="vts", name="vts")
            for sc16 in range(DC):
                vp = pp("vp")
                for dc in range(DC):
                    nc.tensor.matmul(
                        vp[:, :E],
                        xts[:, dc, sc16 * 128:(sc16 + 1) * 128],
                        wvts[:, dc, :], start=(dc == 0), stop=(dc == DC - 1))
                nc.vector.tensor_copy(out=vts[:, sc16, :], in_=vp[:, :E])

            # ---- attention (transposed layout), gated output kept in SBUF ----
            attts = const.tile([128, HPC, S], BF, tag="attts", name="attts")

            # Software-pipelined across (h) blocks within a q-chunk: each
            # block's last AV/sums matmuls and its epilogue are emitted after
            # the NEXT block's first scores/exp, so the PE never idles
            # waiting for the tail exp on ACT.
            pend = None   # deferred tail of the previous block

            def emit_tail_av(t, k):
                # deferred AV/sums for jj_l-1 (k=0) or jj_l (k=1, stop)
                (h, q0, avp, sump, exts_l, s0s, jj_l) = t
                jj = jj_l - 1 + k
                s0 = s0s[k]
                nc.tensor.matmul(
                    avp[:, s0:], vts[:, jj, h * 128:(h + 1) * 128],
                    exts_l[jj % 3][:, s0:], start=False, stop=(k == 1))
                nc.tensor.matmul(
                    sump[:, s0:], ones128[:], exts_l[jj % 3][:, s0:],
                    start=False, stop=(k == 1))

            def emit_tail(t):
                (h, q0, avp, sump, exts_l, s0s, jj_l) = t
                rs = work.tile([1, QCW], F32, tag="rs", bufs=2, name="rs")
                nc.vector.reciprocal(out=rs[:], in_=sump[:])
                # broadcast 1/sum across partitions on the (idle) Pool engine
                # so the epilogue never blocks the PE
                bcb = work.tile([128, QCW], F32, tag="bcb", bufs=2, name="bcb")
                nc.gpsimd.partition_broadcast(bcb[:], rs[:])
                gn = work.tile([128, QCW], BF, tag="gn", bufs=2, name="gn")
                nc.vector.tensor_mul(gn[:], gts[:, h, q0:q0 + QCW], bcb[:])
                nc.vector.tensor_mul(attts[:, h, q0:q0 + QCW], avp[:], gn[:])

            def emit_att_block(h, qc):
                nonlocal pend
                q0 = qc * QCW
                scps = [pp("scp") for _ in range(3)]
                avp = pp("avp")
                sump = psum.tile([1, QCW], F32, tag="pp", bufs=8, name="sump")
                njj = 4 * qc + 4
                exts = [work.tile([128, QCW], BF, tag="ext", bufs=6,
                                  name="ext") for _ in range(3)]

                def s0_of(jj):
                    return max(0, (jj - 4 * qc) * 128)

                def emit_av(jj):
                    s0 = s0_of(jj)
                    nc.tensor.matmul(
                        avp[:, s0:], vts[:, jj, h * 128:(h + 1) * 128],
                        exts[jj % 3][:, s0:],
                        start=(jj == 0), stop=False)
                    nc.tensor.matmul(
                        sump[:, s0:], ones128[:], exts[jj % 3][:, s0:],
                        start=(jj == 0), stop=False)

                # scores run two jj ahead of AV/sums so the PE never
                # waits on the exp->mask chain; the last block's two
                # deferred AV/sums pairs land in this block's jj=0/1
                for jj in range(njj):
                    off = jj - 4 * qc
                    s0 = s0_of(jj)
                    scp = scps[jj % 3]
                    ext = exts[jj % 3]
                    nc.tensor.matmul(
                        scp[:, s0:], kts[:, h, jj * 128:(jj + 1) * 128],
                        qts[:, h, q0 + s0:q0 + QCW], start=True, stop=True)
                    nc.scalar.activation(ext[:, s0:], scp[:, s0:],
                                         AF.Exp, scale=SCALE)
                    if off >= 0:
                        nc.vector.tensor_mul(ext[:, s0:s0 + 128],
                                             ext[:, s0:s0 + 128], tris[:])
                    if pend is not None:
                        if jj == 0:
                            emit_tail_av(pend, 0)
                        elif jj == 1:
                            emit_tail_av(pend, 1)
                            emit_tail(pend)
                            pend = None
                    if jj >= 2:
                        emit_av(jj - 2)
                pend = (h, q0, avp, sump, exts,
                        (s0_of(njj - 2), s0_of(njj - 1)), njj - 1)

            def flush_pend():
                nonlocal pend
                emit_tail_av(pend, 0)
                emit_tail_av(pend, 1)
                emit_tail(pend)
                pend = None

            # ---- per q-chunk: attention for both heads, then the row-
            # parallel o_proj partial for those 512 rows:
            #   y[s, e] = sum_f att[f, s] * o_w[e, f]   (f = local 256)
            # o_proj matmuls and the 4x1MB output DMAs overlap the next
            # q-chunk's attention stream.
            for qc in range(NQC):
                for h in range(HPC):
                    emit_att_block(h, qc)
                flush_pend()
                for i in range(QCW // 128):
                    s0 = qc * QCW + i * 128
                    ys = work.tile([128, D], F32, tag="ys", bufs=2, name="ys")
                    for ec in range(NEC):
                        yp = pp("yp")
                        for f in range(HPC):
                            nc.tensor.matmul(
                                yp[:], attts[:, f, s0:s0 + 128],
                                owts2[:, f, ec * QCW:(ec + 1) * QCW],
                                start=(f == 0), stop=(f == HPC - 1))
                        nc.vector.tensor_copy(
                            out=ys[:, ec * QCW:(ec + 1) * QCW], in_=yp[:])
                    nc.sync.dma_start(yt.ap()[s0:s0 + 128, :], ys[:])

    nc.compile()
    return nc


def _prep_inputs(x, q_w, k_w, v_w, o_w, gate_w, gate_b):
    x = np.asarray(x, dtype=np.float32)
    xt = np.ascontiguousarray(x.reshape(S, D).T).astype(BF16)
    gwt = np.ascontiguousarray(np.asarray(gate_w, np.float32).T).astype(BF16)
    gb = np.asarray(gate_b, np.float32).reshape(HD, 1).copy()
    trim = np.triu(np.ones((128, 128), np.float32)).astype(BF16)
    o_w = np.asarray(o_w, np.float32)
    in_maps = []
    for c in range(N_CORES):
        sl = slice(c * E, (c + 1) * E)
        in_maps.append({
            "xt": xt,
            "wqt": np.ascontiguousarray(np.asarray(q_w, np.float32)[sl, :].T).astype(BF16),
            "wkt": np.ascontiguousarray(np.asarray(k_w, np.float32)[sl, :].T).astype(BF16),
            "wvt": np.ascontiguousarray(np.asarray(v_w, np.float32)[sl, :].T).astype(BF16),
            "owt2": np.ascontiguousarray(o_w[:, sl].T).astype(BF16),
            "gwt": gwt,
            "gb": gb,
            "trim": trim,
        })
    return in_maps


def _run(in_maps, **kwargs):
    if "nc" not in _CACHED:
        _CACHED["nc"] = _build()
    return run_bass_kernel_spmd(_CACHED["nc"], in_maps,
                                core_ids=list(range(N_CORES)), **kwargs)


def kernel(x, q_w, k_w, v_w, o_w, gate_w, gate_b):
    res = _run(_prep_inputs(x, q_w, k_w, v_w, o_w, gate_w, gate_b))
    y = res.results[0]["yt"].astype(np.float64)
    for c in range(1, N_CORES):
        y += res.results[c]["yt"]
    return np.ascontiguousarray(y.astype(np.float32)).reshape(1, S, D)


# revision 7
# speedup vs baseline: 1.3188x; 1.2742x over previous
"""GatedAttention Trainium2 kernel, 8-way tensor-parallel over heads.

Reference computation (B=1, S=2048, D=2048, H=16 heads, Hd=128):
  q,k,v = x @ {q,k,v}_w.T  (per-head split)
  scores = (q @ k.T) / sqrt(Hd), causal mask, softmax
  av = attn @ v
  gate = sigmoid(q @ gate_w.T + gate_b)       (per-head)
  y = concat_heads(av * gate) @ o_w.T

Sharding: 2 heads per core (column-parallel QKV/gate), o_proj ROW-parallel:
each core contracts its own 256 attention-output features against the
matching o_w columns and writes a full [S, D] fp32 partial; the host sums
the 8 partials. No cross-core collectives anywhere, so each core's NEFF
span contains only its own work — immune to launch skew and collective
stalls on the other cores.

All matmuls run on the PE in bf16 with fp32 PSUM accumulation. Softmax runs
without max-subtraction (scores are small by construction); exp row-sums
ride on the PE as M=1 ones-matmuls in the same transposed [j, q] layout, so
no on-chip transposes are needed anywhere. Gate sigmoids are all computed
before attention so the ACT engine loads each activation table once.
o_proj is emitted per q-chunk right after that chunk's attention epilogue,
spreading the 16MB output DMA across the attention stream.
"""

import numpy as np
import ml_dtypes

import concourse.bass as bass
import concourse.mybir as mybir
import concourse.tile as tile
from concourse import bacc
from concourse.bass_utils import run_bass_kernel_spmd

BF16 = ml_dtypes.bfloat16
F32 = mybir.dt.float32
BF = mybir.dt.bfloat16
AF = mybir.ActivationFunctionType

N_CORES = 8
S = 2048          # sequence length
D = 2048          # model dim
H = 16            # total heads
HD = 128          # head dim
HPC = H // N_CORES                   # heads per core: 2
E = HPC * HD                         # 256 local features per core
DC = D // 128                        # 16 contraction chunks
QCW = 512                            # q-chunk width
NQC = S // QCW                       # 4 q-chunks
NEC = D // QCW                       # 4 o_proj output column chunks
SCALE = 1.0 / float(np.sqrt(HD))

_CACHED = {}


def _build():
    nc = bacc.Bacc("TRN2", target_bir_lowering=False, debug=False,
                   num_devices=1, enable_asserts=False)

    xt = nc.dram_tensor("xt", [D, S], BF, kind="ExternalInput")        # x^T
    wqt = nc.dram_tensor("wqt", [D, E], BF, kind="ExternalInput")      # q_w shard^T
    wkt = nc.dram_tensor("wkt", [D, E], BF, kind="ExternalInput")
    wvt = nc.dram_tensor("wvt", [D, E], BF, kind="ExternalInput")
    owt2 = nc.dram_tensor("owt2", [E, D], BF, kind="ExternalInput")    # o_w cols^T
    gwt = nc.dram_tensor("gwt", [HD, HD], BF, kind="ExternalInput")    # gate_w^T
    gb = nc.dram_tensor("gb", [HD, 1], F32, kind="ExternalInput")      # gate bias
    trim = nc.dram_tensor("trim", [128, 128], BF, kind="ExternalInput")
    yt = nc.dram_tensor("yt", [S, D], BF, kind="ExternalOutput")       # partial y

    with tile.TileContext(nc) as tc:
        with tc.tile_pool(name="const", bufs=1) as const, \
             tc.tile_pool(name="work", bufs=2) as work, \
             tc.tile_pool(name="psum", bufs=1, space="PSUM") as psum:

            def pp(name):
                return psum.tile([128, QCW], F32, tag="pp", bufs=8, name=name)

            # ---- input loads (few big DMAs; xts chunked to feed the
            #      dc-synchronized projection loop) ----
            wqts = const.tile([128, DC, E], BF, tag="wqts", name="wqts")
            wkts = const.tile([128, DC, E], BF, tag="wkts", name="wkts")
            xts = const.tile([128, DC, S], BF, tag="big", name="xts")

            def _ldw(dst, src, c0, c1):
                nc.sync.dma_start(
                    dst[:, c0:c1, :],
                    src.ap()[c0 * 128:c1 * 128, :]
                       .rearrange("(c p) e -> p c e", p=128))

            # interleave weight chunks with the x chunks so the transfer
            # stream stays just ahead of group A's dc-ordered consumption;
            # the first Q/K matmuls need only wq/wk dc=0 + x chunk 0, so
            # those tiny loads go first and the PE starts ~5us in.
            def _ldx(d0, d1):
                nc.sync.dma_start(
                    xts[:, d0:d1, :],
                    xt.ap()[d0 * 128:d1 * 128, :]
                      .rearrange("(c p) s -> p c s", p=128))

            _ldw(wqts, wqt, 0, 1)
            _ldx(0, 1)
            _ldw(wkts, wkt, 0, 1)
            _ldw(wqts, wqt, 1, 4)
            _ldw(wkts, wkt, 1, 4)
            _ldx(1, 2)
            _ldw(wqts, wqt, 4, 8)
            _ldw(wkts, wkt, 4, 8)
            for d in range(2, 6):
                _ldx(d, d + 1)
            _ldw(wqts, wqt, 8, 16)
            _ldx(6, 7)
            _ldw(wkts, wkt, 8, 16)
            _ldx(7, 8)
            for k in range(4, 8):
                _ldx(2 * k, 2 * k + 2)

            gwts = const.tile([HD, HD], BF, tag="gwts", name="gwts")
            gbs = const.tile([HD, 1], F32, tag="gbs", name="gbs")
            tris = const.tile([128, 128], BF, tag="tris", name="tris")
            ones_mat = const.tile([128, 128], BF, tag="ones_mat",
                                  name="ones_mat")
            nc.sync.dma_start(gwts[:], gwt.ap())
            nc.sync.dma_start(gbs[:], gb.ap())
            nc.sync.dma_start(tris[:], trim.ap())
            nc.vector.memset(ones_mat[:], 1.0)

            wvts = const.tile([128, DC, E], BF, tag="wvts", name="wvts")
            nc.sync.dma_start(wvts[:], wvt.ap().rearrange("(c p) e -> p c e", p=128))

            # ---- projections ----
            # Q^T, K^T: [e(2x128), s].  Groups of 8 PSUM banks, dc-inner so
            # PE work tracks the streaming xts chunks.
            qts = const.tile([128, HPC, S], BF, tag="qts", name="qts")
            kts = const.tile([128, HPC, S], BF, tag="kts", name="kts")

            # ec=0: dc-inner across 8 psums so PE work tracks streaming xts
            # chunks.  ec=1: slot-major (xts resident), each chain overlaps
            # the previous psum's copy.
            qps = [pp("qp") for _ in range(NQC)]
            kps = [pp("kp") for _ in range(NQC)]
            for dc in range(DC):
                st = (dc == 0)
                sp = (dc == DC - 1)
                for sc in range(NQC):
                    nc.tensor.matmul(
                        qps[sc][:], wqts[:, dc, 0:128],
                        xts[:, dc, sc * QCW:(sc + 1) * QCW], start=st, stop=sp)
                for sc in range(NQC):
                    nc.tensor.matmul(
                        kps[sc][:], wkts[:, dc, 0:128],
                        xts[:, dc, sc * QCW:(sc + 1) * QCW], start=st, stop=sp)
            for sc in range(NQC):
                nc.vector.tensor_copy(
                    out=qts[:, 0, sc * QCW:(sc + 1) * QCW], in_=qps[sc][:])
                nc.vector.tensor_copy(
                    out=kts[:, 0, sc * QCW:(sc + 1) * QCW], in_=kps[sc][:])
            for wts, outts in ((wqts, qts), (wkts, kts)):
                for sc in range(NQC):
                    ppt = pp("qp")
                    for dc in range(DC):
                        nc.tensor.matmul(
                            ppt[:], wts[:, dc, 128:256],
                            xts[:, dc, sc * QCW:(sc + 1) * QCW],
                            start=(dc == 0), stop=(dc == DC - 1))
                    nc.vector.tensor_copy(
                        out=outts[:, 1, sc * QCW:(sc + 1) * QCW], in_=ppt[:])

            # o_proj weights (row-parallel slice): [f(2x128), e(2048)]
            owts2 = const.tile([128, HPC, D], BF, tag="owts2", name="owts2")
            nc.sync.dma_start(owts2[:], owt2.ap().rearrange("(c p) e -> p c e", p=128))

            # gates for both heads, before the V projection so the sigmoid
            # table load and ACT latency hide behind V's matmuls
            gts = const.tile([128, HPC, S], BF, tag="gts", name="gts")
            for h in range(HPC):
                for qc in range(NQC):
                    gp = pp("gp")
                    nc.tensor.matmul(gp[:], gwts[:],
                                     qts[:, h, qc * QCW:(qc + 1) * QCW],
                                     start=True, stop=True)
                    nc.scalar.activation(gts[:, h, qc * QCW:(qc + 1) * QCW],
                                         gp[:], AF.Sigmoid, bias=gbs[:, 0:1])

            # V: [s(16x128), e] natural layout.  Slot-major (xts is fully
            # resident by now): each psum's 16-matmul chain runs while the
            # previous psum's copy drains, so group boundaries don't stall.
            vts = const.tile([128, DC, E], BF, tag="vts", name="vts")
            for sc16 in range(DC):
                vp = pp("vp")
                for dc in range(DC):
                    nc.tensor.matmul(
                        vp[:, :E],
                        xts[:, dc, sc16 * 128:(sc16 + 1) * 128],
                        wvts[:, dc, :], start=(dc == 0), stop=(dc == DC - 1))
                nc.vector.tensor_copy(out=vts[:, sc16, :], in_=vp[:, :E])

            # ---- attention (transposed layout), gated output kept in SBUF ----
            attts = const.tile([128, HPC, S], BF, tag="attts", name="attts")

            # Software-pipelined across (h) blocks within a q-chunk: each
            # block's last AV/sums matmuls and its epilogue are emitted after
            # the NEXT block's first scores/exp, so the PE never idles
            # waiting for the tail exp on ACT.
            pend = None   # deferred tail of the previous block

            def emit_tail_av(t, k):
                # deferred AV/sums for jj_l-1 (k=0) or jj_l (k=1, stop)
                (h, q0, avp, sump, exts_l, s0s, jj_l) = t
                jj = jj_l - 1 + k
                s0 = s0s[k]
                nc.tensor.matmul(
                    avp[:, s0:], vts[:, jj, h * 128:(h + 1) * 128],
                    exts_l[jj % 3][:, s0:], start=False, stop=(k == 1))
                nc.tensor.matmul(
                    sump[:, s0:], ones_mat[:], exts_l[jj % 3][:, s0:],
                    start=False, stop=(k == 1))

            def emit_tail(t):
                # sump carries sum_k exp replicated across all 128
                # partitions (ones-matrix matmul), so the reciprocal runs
                # 128-way parallel and no partition broadcast is needed.
                (h, q0, avp, sump, exts_l, s0s, jj_l) = t
                bcb = work.tile([128, QCW], F32, tag="bcb", bufs=2, name="bcb")
                nc.vector.reciprocal(out=bcb[:], in_=sump[:])
                gn = work.tile([128, QCW], BF, tag="gn", bufs=2, name="gn")
                nc.vector.tensor_mul(gn[:], gts[:, h, q0:q0 + QCW], bcb[:])
                nc.vector.tensor_mul(attts[:, h, q0:q0 + QCW], avp[:], gn[:])

            def emit_att_block(h, qc):
                nonlocal pend
                q0 = qc * QCW
                scps = [pp("scp") for _ in range(3)]
                avp = pp("avp")
                sump = pp("sump")
                njj = 4 * qc + 4
                exts = [work.tile([128, QCW], BF, tag="ext", bufs=6,
                                  name="ext") for _ in range(3)]

                def s0_of(jj):
                    return max(0, (jj - 4 * qc) * 128)

                def emit_av(jj):
                    s0 = s0_of(jj)
                    nc.tensor.matmul(
                        avp[:, s0:], vts[:, jj, h * 128:(h + 1) * 128],
                        exts[jj % 3][:, s0:],
                        start=(jj == 0), stop=False)
                    nc.tensor.matmul(
                        sump[:, s0:], ones_mat[:], exts[jj % 3][:, s0:],
                        start=(jj == 0), stop=False)

                # scores run two jj ahead of AV/sums so the PE never
                # waits on the exp->mask chain; the last block's two
                # deferred AV/sums pairs land in this block's jj=0/1
                for jj in range(njj):
                    off = jj - 4 * qc
                    s0 = s0_of(jj)
                    scp = scps[jj % 3]
                    ext = exts[jj % 3]
                    nc.tensor.matmul(
                        scp[:, s0:], kts[:, h, jj * 128:(jj + 1) * 128],
                        qts[:, h, q0 + s0:q0 + QCW], start=True, stop=True)
                    nc.scalar.activation(ext[:, s0:], scp[:, s0:],
                                         AF.Exp, scale=SCALE)
                    if off >= 0:
                        nc.vector.tensor_mul(ext[:, s0:s0 + 128],
                                             ext[:, s0:s0 + 128], tris[:])
                    if pend is not None:
                        if jj == 0:
                            emit_tail_av(pend, 0)
                        elif jj == 1:
                            emit_tail_av(pend, 1)
                            emit_tail(pend)
                            pend = None
                    if jj >= 2:
                        emit_av(jj - 2)
                pend = (h, q0, avp, sump, exts,
                        (s0_of(njj - 2), s0_of(njj - 1)), njj - 1)

            def flush_pend():
                nonlocal pend
                emit_tail_av(pend, 0)
                emit_tail_av(pend, 1)
                emit_tail(pend)
                pend = None

            # ---- per q-chunk: attention for both heads, then the row-
            # parallel o_proj partial for those 512 rows:
            #   y[s, e] = sum_f att[f, s] * o_w[e, f]   (f = local 256)
            # o_proj matmuls and the 4x1MB output DMAs overlap the next
            # q-chunk's attention stream.
            for qc in range(NQC):
                for h in range(HPC):
                    emit_att_block(h, qc)
                flush_pend()
                # o_proj: per s-slice emit all head-0 partials first (head 0
                # flushed a block earlier, so those matmuls never wait on the
                # epilogue above), then the head-1 accumulates + copies.
                for i in range(QCW // 128):
                    s0 = qc * QCW + i * 128
                    ys = work.tile([128, D], BF, tag="ys", bufs=2, name="ys")
                    yps = [pp("yp") for _ in range(NEC)]
                    for ec in range(NEC):
                        nc.tensor.matmul(
                            yps[ec][:], attts[:, 0, s0:s0 + 128],
                            owts2[:, 0, ec * QCW:(ec + 1) * QCW],
                            start=True, stop=False)
                    for ec in range(NEC):
                        nc.tensor.matmul(
                            yps[ec][:], attts[:, 1, s0:s0 + 128],
                            owts2[:, 1, ec * QCW:(ec + 1) * QCW],
                            start=False, stop=True)
                        nc.vector.tensor_copy(
                            out=ys[:, ec * QCW:(ec + 1) * QCW], in_=yps[ec][:])
                    nc.sync.dma_start(yt.ap()[s0:s0 + 128, :], ys[:])

    nc.compile()
    return nc


def _prep_inputs(x, q_w, k_w, v_w, o_w, gate_w, gate_b):
    x = np.asarray(x, dtype=np.float32)
    xt = np.ascontiguousarray(x.reshape(S, D).T).astype(BF16)
    gwt = np.ascontiguousarray(np.asarray(gate_w, np.float32).T).astype(BF16)
    gb = np.asarray(gate_b, np.float32).reshape(HD, 1).copy()
    trim = np.triu(np.ones((128, 128), np.float32)).astype(BF16)
    o_w = np.asarray(o_w, np.float32)
    in_maps = []
    for c in range(N_CORES):
        sl = slice(c * E, (c + 1) * E)
        in_maps.append({
            "xt": xt,
            "wqt": np.ascontiguousarray(np.asarray(q_w, np.float32)[sl, :].T).astype(BF16),
            "wkt": np.ascontiguousarray(np.asarray(k_w, np.float32)[sl, :].T).astype(BF16),
            "wvt": np.ascontiguousarray(np.asarray(v_w, np.float32)[sl, :].T).astype(BF16),
            "owt2": np.ascontiguousarray(o_w[:, sl].T).astype(BF16),
            "gwt": gwt,
            "gb": gb,
            "trim": trim,
        })
    return in_maps


def _run(in_maps, **kwargs):
    if "nc" not in _CACHED:
        _CACHED["nc"] = _build()
    return run_bass_kernel_spmd(_CACHED["nc"], in_maps,
                                core_ids=list(range(N_CORES)), **kwargs)


def kernel(x, q_w, k_w, v_w, o_w, gate_w, gate_b):
    res = _run(_prep_inputs(x, q_w, k_w, v_w, o_w, gate_w, gate_b))
    y = res.results[0]["yt"].astype(np.float32)
    for c in range(1, N_CORES):
        y += res.results[c]["yt"].astype(np.float32)
    return np.ascontiguousarray(y).reshape(1, S, D)


# revision 9
# speedup vs baseline: 1.4674x; 1.1126x over previous
"""GatedAttention Trainium2 kernel, 8-way tensor-parallel over heads.

Reference computation (B=1, S=2048, D=2048, H=16 heads, Hd=128):
  q,k,v = x @ {q,k,v}_w.T  (per-head split)
  scores = (q @ k.T) / sqrt(Hd), causal mask, softmax
  av = attn @ v
  gate = sigmoid(q @ gate_w.T + gate_b)       (per-head)
  y = concat_heads(av * gate) @ o_w.T

Sharding: 2 heads per core (column-parallel QKV/gate), o_proj ROW-parallel:
each core contracts its own 256 attention-output features against the
matching o_w columns and writes a full [S, D] fp32 partial; the host sums
the 8 partials. No cross-core collectives anywhere, so each core's NEFF
span contains only its own work — immune to launch skew and collective
stalls on the other cores.

All matmuls run on the PE in bf16 with fp32 PSUM accumulation. Softmax runs
without max-subtraction (scores are small by construction); exp row-sums
ride on the PE as M=1 ones-matmuls in the same transposed [j, q] layout, so
no on-chip transposes are needed anywhere. Gate sigmoids are all computed
before attention so the ACT engine loads each activation table once.
o_proj is emitted per q-chunk right after that chunk's attention epilogue,
spreading the 16MB output DMA across the attention stream.
"""

import numpy as np
import ml_dtypes

import concourse.bass as bass
import concourse.mybir as mybir
import concourse.tile as tile
from concourse import bacc
from concourse.bass_utils import run_bass_kernel_spmd

BF16 = ml_dtypes.bfloat16
F32 = mybir.dt.float32
BF = mybir.dt.bfloat16
AF = mybir.ActivationFunctionType

N_CORES = 8
S = 2048          # sequence length
D = 2048          # model dim
H = 16            # total heads
HD = 128          # head dim
HPC = H // N_CORES                   # heads per core: 2
E = HPC * HD                         # 256 local features per core
DC = D // 128                        # 16 contraction chunks
QCW = 512                            # q-chunk width
NQC = S // QCW                       # 4 q-chunks
NEC = D // QCW                       # 4 o_proj output column chunks
SCALE = 1.0 / float(np.sqrt(HD))

_CACHED = {}


def _build():
    nc = bacc.Bacc("TRN2", target_bir_lowering=False, debug=False,
                   num_devices=1, enable_asserts=False)

    xt = nc.dram_tensor("xt", [D, S], BF, kind="ExternalInput")        # x^T
    wqt = nc.dram_tensor("wqt", [D, E], BF, kind="ExternalInput")      # q_w shard^T
    wkt = nc.dram_tensor("wkt", [D, E], BF, kind="ExternalInput")
    wvt = nc.dram_tensor("wvt", [D, E], BF, kind="ExternalInput")
    owt2 = nc.dram_tensor("owt2", [E, D], BF, kind="ExternalInput")    # o_w cols^T
    gwt = nc.dram_tensor("gwt", [HD, HD], BF, kind="ExternalInput")    # gate_w^T
    gb = nc.dram_tensor("gb", [HD, 1], F32, kind="ExternalInput")      # gate bias
    trim = nc.dram_tensor("trim", [128, 128], BF, kind="ExternalInput")
    yt = nc.dram_tensor("yt", [S, D], BF, kind="ExternalOutput")       # partial y

    with tile.TileContext(nc) as tc:
        with tc.tile_pool(name="const", bufs=1) as const, \
             tc.tile_pool(name="work", bufs=2) as work, \
             tc.tile_pool(name="psum", bufs=1, space="PSUM") as psum:

            def pp(name):
                return psum.tile([128, QCW], F32, tag="pp", bufs=8, name=name)

            # ---- input loads (few big DMAs; xts chunked to feed the
            #      dc-synchronized projection loop) ----
            wqts = const.tile([128, DC, E], BF, tag="wqts", name="wqts")
            wkts = const.tile([128, DC, E], BF, tag="wkts", name="wkts")
            xts = const.tile([128, DC, S], BF, tag="big", name="xts")

            def _ldw(dst, src, c0, c1):
                nc.sync.dma_start(
                    dst[:, c0:c1, :],
                    src.ap()[c0 * 128:c1 * 128, :]
                       .rearrange("(c p) e -> p c e", p=128))

            # interleave weight chunks with the x chunks so the transfer
            # stream stays just ahead of group A's dc-ordered consumption;
            # the first Q/K matmuls need only wq/wk dc=0 + x chunk 0, so
            # those tiny loads go first and the PE starts ~5us in.
            def _ldx(d0, d1):
                nc.sync.dma_start(
                    xts[:, d0:d1, :],
                    xt.ap()[d0 * 128:d1 * 128, :]
                      .rearrange("(c p) s -> p c s", p=128))

            _ldw(wqts, wqt, 0, 1)
            _ldx(0, 1)
            _ldw(wkts, wkt, 0, 1)
            _ldw(wqts, wqt, 1, 4)
            _ldw(wkts, wkt, 1, 4)
            _ldx(1, 2)
            _ldw(wqts, wqt, 4, 8)
            _ldw(wkts, wkt, 4, 8)
            for d in range(2, 6):
                _ldx(d, d + 1)
            _ldw(wqts, wqt, 8, 16)
            _ldx(6, 7)
            _ldw(wkts, wkt, 8, 16)
            _ldx(7, 8)
            for k in range(4, 8):
                _ldx(2 * k, 2 * k + 2)

            gwts = const.tile([HD, HD], BF, tag="gwts", name="gwts")
            gbs = const.tile([HD, 1], F32, tag="gbs", name="gbs")
            tris = const.tile([128, 128], BF, tag="tris", name="tris")
            ones_mat = const.tile([128, 128], BF, tag="ones_mat",
                                  name="ones_mat")
            nc.sync.dma_start(gwts[:], gwt.ap())
            nc.sync.dma_start(gbs[:], gb.ap())
            nc.sync.dma_start(tris[:], trim.ap())
            nc.vector.memset(ones_mat[:], 1.0)

            wvts = const.tile([128, DC, E], BF, tag="wvts", name="wvts")
            nc.sync.dma_start(wvts[:], wvt.ap().rearrange("(c p) e -> p c e", p=128))

            # ---- projections ----
            # Q^T, K^T: [e(2x128), s].  Groups of 8 PSUM banks, dc-inner so
            # PE work tracks the streaming xts chunks.
            qts = const.tile([128, HPC, S], BF, tag="qts", name="qts")
            kts = const.tile([128, HPC, S], BF, tag="kts", name="kts")

            # ec=0: dc-inner across 8 psums so PE work tracks streaming xts
            # chunks.  ec=1: slot-major (xts resident), each chain overlaps
            # the previous psum's copy.
            qps = [pp("qp") for _ in range(NQC)]
            kps = [pp("kp") for _ in range(NQC)]
            for dc in range(DC):
                st = (dc == 0)
                sp = (dc == DC - 1)
                for sc in range(NQC):
                    nc.tensor.matmul(
                        qps[sc][:], wqts[:, dc, 0:128],
                        xts[:, dc, sc * QCW:(sc + 1) * QCW], start=st, stop=sp)
                for sc in range(NQC):
                    nc.tensor.matmul(
                        kps[sc][:], wkts[:, dc, 0:128],
                        xts[:, dc, sc * QCW:(sc + 1) * QCW], start=st, stop=sp)
            for sc in range(NQC):
                nc.vector.tensor_copy(
                    out=qts[:, 0, sc * QCW:(sc + 1) * QCW], in_=qps[sc][:])
                nc.vector.tensor_copy(
                    out=kts[:, 0, sc * QCW:(sc + 1) * QCW], in_=kps[sc][:])
            for wts, outts in ((wqts, qts), (wkts, kts)):
                for sc in range(NQC):
                    ppt = pp("qp")
                    for dc in range(DC):
                        nc.tensor.matmul(
                            ppt[:], wts[:, dc, 128:256],
                            xts[:, dc, sc * QCW:(sc + 1) * QCW],
                            start=(dc == 0), stop=(dc == DC - 1))
                    nc.vector.tensor_copy(
                        out=outts[:, 1, sc * QCW:(sc + 1) * QCW], in_=ppt[:])

            # o_proj weights (row-parallel slice): [f(2x128), e(2048)]
            owts2 = const.tile([128, HPC, D], BF, tag="owts2", name="owts2")
            nc.sync.dma_start(owts2[:], owt2.ap().rearrange("(c p) e -> p c e", p=128))

            # gates for both heads, before the V projection so the sigmoid
            # table load and ACT latency hide behind V's matmuls
            gts = const.tile([128, HPC, S], BF, tag="gts", name="gts")
            for h in range(HPC):
                for qc in range(NQC):
                    gp = pp("gp")
                    nc.tensor.matmul(gp[:], gwts[:],
                                     qts[:, h, qc * QCW:(qc + 1) * QCW],
                                     start=True, stop=True)
                    nc.scalar.activation(gts[:, h, qc * QCW:(qc + 1) * QCW],
                                         gp[:], AF.Sigmoid, bias=gbs[:, 0:1])

            # V: [s(16x128), e] natural layout.  Slot-major (xts is fully
            # resident by now): each psum's 16-matmul chain runs while the
            # previous psum's copy drains, so group boundaries don't stall.
            vts = const.tile([128, DC, E], BF, tag="vts", name="vts")
            for sc16 in range(DC):
                vp = pp("vp")
                for dc in range(DC):
                    nc.tensor.matmul(
                        vp[:, :E],
                        xts[:, dc, sc16 * 128:(sc16 + 1) * 128],
                        wvts[:, dc, :], start=(dc == 0), stop=(dc == DC - 1))
                nc.vector.tensor_copy(out=vts[:, sc16, :], in_=vp[:, :E])

            # ---- attention (transposed layout), gated output kept in SBUF ----
            attts = const.tile([128, HPC, S], BF, tag="attts", name="attts")

            # Software-pipelined across (h) blocks within a q-chunk: each
            # block's last AV/sums matmuls and its epilogue are emitted after
            # the NEXT block's first scores/exp, so the PE never idles
            # waiting for the tail exp on ACT.
            pend = None   # deferred tail of the previous block

            def emit_tail_av(t, k):
                # deferred AV/sums for jj_l-1 (k=0) or jj_l (k=1, stop).
                # sums goes first so the reciprocal can start one matmul
                # earlier than the gated-output multiply needs the AV.
                (h, q0, avp, sump, exts_l, s0s, jj_l) = t
                jj = jj_l - 1 + k
                s0 = s0s[k]
                nc.tensor.matmul(
                    sump[:, s0:], ones_mat[:], exts_l[jj % 3][:, s0:],
                    start=False, stop=(k == 1))
                nc.tensor.matmul(
                    avp[:, s0:], vts[:, jj, h * 128:(h + 1) * 128],
                    exts_l[jj % 3][:, s0:], start=False, stop=(k == 1))

            def emit_tail(t):
                # sump carries sum_k exp replicated across all 128
                # partitions (ones-matrix matmul), so 1/sum runs 128-way
                # parallel; approx_fast (~18 bits) is plenty for softmax
                # normalization and ~5x faster than full reciprocal.
                (h, q0, avp, sump, exts_l, s0s, jj_l) = t
                bcb = work.tile([128, QCW], F32, tag="bcb", bufs=2, name="bcb")
                nc.vector.reciprocal_approx_fast(out=bcb[:], in_=sump[:])
                avg = work.tile([128, QCW], BF, tag="avg", bufs=2, name="avg")
                nc.vector.tensor_mul(avg[:], avp[:], gts[:, h, q0:q0 + QCW])
                nc.vector.tensor_mul(attts[:, h, q0:q0 + QCW], avg[:], bcb[:])

            def emit_att_block(h, qc):
                nonlocal pend
                q0 = qc * QCW
                scps = [pp("scp") for _ in range(3)]
                avp = pp("avp")
                sump = pp("sump")
                njj = 4 * qc + 4
                exts = [work.tile([128, QCW], BF, tag="ext", bufs=6,
                                  name="ext") for _ in range(3)]

                def s0_of(jj):
                    return max(0, (jj - 4 * qc) * 128)

                def emit_av(jj):
                    s0 = s0_of(jj)
                    nc.tensor.matmul(
                        avp[:, s0:], vts[:, jj, h * 128:(h + 1) * 128],
                        exts[jj % 3][:, s0:],
                        start=(jj == 0), stop=False)
                    nc.tensor.matmul(
                        sump[:, s0:], ones_mat[:], exts[jj % 3][:, s0:],
                        start=(jj == 0), stop=False)

                # scores run two jj ahead of AV/sums so the PE never
                # waits on the exp->mask chain; the last block's two
                # deferred AV/sums pairs land in this block's jj=0/1
                for jj in range(njj):
                    off = jj - 4 * qc
                    s0 = s0_of(jj)
                    scp = scps[jj % 3]
                    ext = exts[jj % 3]
                    nc.tensor.matmul(
                        scp[:, s0:], kts[:, h, jj * 128:(jj + 1) * 128],
                        qts[:, h, q0 + s0:q0 + QCW], start=True, stop=True)
                    nc.scalar.activation(ext[:, s0:], scp[:, s0:],
                                         AF.Exp, scale=SCALE)
                    if off >= 0:
                        nc.vector.tensor_mul(ext[:, s0:s0 + 128],
                                             ext[:, s0:s0 + 128], tris[:])
                    if pend is not None:
                        if jj == 0:
                            emit_tail_av(pend, 0)
                        elif jj == 1:
                            emit_tail_av(pend, 1)
                            emit_tail(pend)
                            pend = None
                    if jj >= 2:
                        emit_av(jj - 2)
                pend = (h, q0, avp, sump, exts,
                        (s0_of(njj - 2), s0_of(njj - 1)), njj - 1)

            def flush_pend():
                nonlocal pend
                emit_tail_av(pend, 0)
                emit_tail_av(pend, 1)
                emit_tail(pend)
                pend = None

            # ---- per q-chunk: attention for both heads, then the row-
            # parallel o_proj partial for those 512 rows:
            #   y[s, e] = sum_f att[f, s] * o_w[e, f]   (f = local 256)
            # o_proj matmuls and the 4x1MB output DMAs overlap the next
            # q-chunk's attention stream.
            for qc in range(NQC):
                for h in range(HPC):
                    emit_att_block(h, qc)
                flush_pend()
                # o_proj: per s-slice emit all head-0 partials first (head 0
                # flushed a block earlier, so those matmuls never wait on the
                # epilogue above), then the head-1 accumulates + copies.
                for i in range(QCW // 128):
                    s0 = qc * QCW + i * 128
                    ys = work.tile([128, D], BF, tag="ys", bufs=2, name="ys")
                    yps = [pp("yp") for _ in range(NEC)]
                    for ec in range(NEC):
                        nc.tensor.matmul(
                            yps[ec][:], attts[:, 0, s0:s0 + 128],
                            owts2[:, 0, ec * QCW:(ec + 1) * QCW],
                            start=True, stop=False)
                    for ec in range(NEC):
                        nc.tensor.matmul(
                            yps[ec][:], attts[:, 1, s0:s0 + 128],
                            owts2[:, 1, ec * QCW:(ec + 1) * QCW],
                            start=False, stop=True)
                        # alternate DVE/ACT so neither engine paces the
                        # o_proj stream (PE is ~1.7us/slice, one engine's
                        # 4 copies would be ~2.6us)
                        if ec % 2 == 0:
                            nc.vector.tensor_copy(
                                out=ys[:, ec * QCW:(ec + 1) * QCW],
                                in_=yps[ec][:])
                        else:
                            nc.scalar.activation(
                                ys[:, ec * QCW:(ec + 1) * QCW], yps[ec][:],
                                AF.Copy)
                    nc.sync.dma_start(yt.ap()[s0:s0 + 128, :], ys[:])

    nc.compile()
    return nc


def _prep_inputs(x, q_w, k_w, v_w, o_w, gate_w, gate_b):
    x = np.asarray(x, dtype=np.float32)
    xt = np.ascontiguousarray(x.reshape(S, D).T).astype(BF16)
    gwt = np.ascontiguousarray(np.asarray(gate_w, np.float32).T).astype(BF16)
    gb = np.asarray(gate_b, np.float32).reshape(HD, 1).copy()
    trim = np.triu(np.ones((128, 128), np.float32)).astype(BF16)
    o_w = np.asarray(o_w, np.float32)
    in_maps = []
    for c in range(N_CORES):
        sl = slice(c * E, (c + 1) * E)
        in_maps.append({
            "xt": xt,
            "wqt": np.ascontiguousarray(np.asarray(q_w, np.float32)[sl, :].T).astype(BF16),
            "wkt": np.ascontiguousarray(np.asarray(k_w, np.float32)[sl, :].T).astype(BF16),
            "wvt": np.ascontiguousarray(np.asarray(v_w, np.float32)[sl, :].T).astype(BF16),
            "owt2": np.ascontiguousarray(o_w[:, sl].T).astype(BF16),
            "gwt": gwt,
            "gb": gb,
            "trim": trim,
        })
    return in_maps


def _run(in_maps, **kwargs):
    if "nc" not in _CACHED:
        _CACHED["nc"] = _build()
    return run_bass_kernel_spmd(_CACHED["nc"], in_maps,
                                core_ids=list(range(N_CORES)), **kwargs)


def kernel(x, q_w, k_w, v_w, o_w, gate_w, gate_b):
    res = _run(_prep_inputs(x, q_w, k_w, v_w, o_w, gate_w, gate_b))
    y = res.results[0]["yt"].astype(np.float32)
    for c in range(1, N_CORES):
        y += res.results[c]["yt"].astype(np.float32)
    return np.ascontiguousarray(y).reshape(1, S, D)
